# revision 50
# baseline (speedup 1.0000x reference)
"""Trainium2 Bass kernel for nn_Attention (B=2,T=8,N=512,C=768,H=12).

Strategy: data-parallel over the 16 (b,t) slices -> 2 slices per core, 8 cores.
All weight/mask transposes are done on host. On-chip per slice:
  xT = transpose(x)                      (PE transposes, 128x128 tiles)
  qkvT[q,k] = W_qk @ xT                  ([d, n] layout; scale folded into Wq on host)
  v = xT.T @ W_v                         ([token, d] layout)
  ST[m,n] = kT.T @ qT ; P = exp(ST + maskT)   (no max-subtraction: logits bounded)
  OT'[0:64] = v_h.T @ P (PV), OT'[64] = row-sums l (ones column in v tiles)
  outT[c,n] = OT' * broadcast(1/l)       (ones-matmul broadcast of recip row)
  y = outT.T @ proj_wT                   ([n, C] layout, DMA out)
All matmuls run in float32r (full PE rate at N>=256).
"""
import sys

sys.path.insert(0, "/opt/trn_rl_repo")

import numpy as np
import concourse.bacc as bacc
import concourse.mybir as mybir
import concourse.tile as tile
from concourse.bass_utils import run_bass_kernel_spmd
from concourse.masks import make_identity

B, T, N, C = 2, 8, 512, 768
H = 12
Dh = C // H            # 64
SL = 2                 # slices per core
NCORES = 8
NC4 = N // 128         # 4 n-chunks
CC6 = C // 128         # 6 c-chunks
F32 = mybir.dt.float32
F32R = mybir.dt.float32r

_cache = {}


def build_nc():
    nc = bacc.Bacc()
    xs = nc.dram_tensor("xs", [SL, N, C], F32R, kind="ExternalInput")
    qkv_wTqk = nc.dram_tensor("qkv_wTqk", [C, 2 * C], F32R, kind="ExternalInput")
    qkv_wTv = nc.dram_tensor("qkv_wTv", [C, C], F32R, kind="ExternalInput")
    proj_wT = nc.dram_tensor("proj_wT", [C, C], F32R, kind="ExternalInput")
    maskT = nc.dram_tensor("maskT", [N, N], F32R, kind="ExternalInput")
    y = nc.dram_tensor("y", [SL, N, C], F32, kind="ExternalOutput")

    with tile.TileContext(nc) as tc:
        with (
            tc.tile_pool(name="wpool", bufs=1) as wpool,
            tc.tile_pool(name="sb", bufs=1) as sb,
            tc.tile_pool(name="ps", bufs=1, space="PSUM") as ps,
        ):
            # ---- persistent weights ----
            qkw = [wpool.tile([128, 2 * C], F32R, tag=f"qkw{cc}", name=f"qkw{cc}") for cc in range(CC6)]
            vw = [wpool.tile([128, C], F32R, tag=f"vw{cc}", name=f"vw{cc}") for cc in range(CC6)]
            projw = [wpool.tile([128, C], F32R, tag=f"projw{cc}", name=f"projw{cc}") for cc in range(CC6)]
            maskt = [wpool.tile([128, N], F32R, tag=f"maskt{mc}", name=f"maskt{mc}") for mc in range(NC4)]
            def emit_weight_dmas():
                for cc in range(CC6):
                    eng = nc.gpsimd if cc % 2 == 0 else nc.scalar
                    eng.dma_start(vw[cc][:], qkv_wTv[128 * cc:128 * (cc + 1), :])
                for cc in range(CC6):
                    eng = (nc.gpsimd, nc.scalar)[cc % 2]
                    eng.dma_start(qkw[cc][:], qkv_wTqk[128 * cc:128 * (cc + 1), :])
                for mc in range(NC4):
                    nc.sync.dma_start(maskt[mc][:], maskT[128 * mc:128 * (mc + 1), :])

            def emit_projw_dmas():
                for cc in range(CC6):
                    nc.gpsimd.dma_start(projw[cc][:], proj_wT[128 * cc:128 * (cc + 1), :])
            identf = wpool.tile([128, 128], F32, tag="identf")
            make_identity(nc, identf[:])
            ident = wpool.tile([128, 128], F32R, tag="ident")
            nc.vector.tensor_copy(ident[:], identf[:])
            onesf = wpool.tile([128, Dh], F32, tag="onesf")
            nc.gpsimd.memset(onesf[:], 1.0)

            xTs = [[None] * CC6 for _ in range(SL)]
            vsbs = [[None] * NC4 for _ in range(SL)]
            qks = [[None] * (2 * CC6) for _ in range(SL)]
            outTs = [[None] * CC6 for _ in range(SL)]

            def get(lst, i, mk):
                if lst[i] is None:
                    lst[i] = mk()
                return lst[i]

            def emit_transpose(s, n4):
                # one contiguous block DMA, then transpose 6 column chunks
                xblk = sb.tile([128, C], F32R, tag="xin", name=f"xblk{s}_{n4}", bufs=3)
                nc.sync.dma_start(xblk[:], xs[s, 128 * n4:128 * (n4 + 1), :])
                for cc in range(CC6):
                    xT = get(xTs[s], cc, lambda cc=cc: sb.tile(
                        [128, N], F32R, tag="xT", name=f"xT_s{s}_{cc}", bufs=8))
                    pt = ps.tile([128, 128], F32R, tag="ps1", name=f"pt{s}_{n4}_{cc}", bufs=8)
                    nc.tensor.transpose(pt[:], xblk[:, 128 * cc:128 * (cc + 1)], ident[:])
                    nc.vector.tensor_copy(xT[:, 128 * n4:128 * (n4 + 1)], pt[:])

            def emit_v(s, n4):
                xT = xTs[s]
                vsb = get(vsbs[s], n4, lambda: sb.tile(
                    [128, H * (Dh + 1)], F32R, tag="vsb", name=f"vsb_s{s}_{n4}", bufs=8))
                pva = ps.tile([128, 512], F32, tag="ps1", name=f"pva{s}_{n4}", bufs=8)
                pvb = ps.tile([128, 256], F32, tag="ps1", name=f"pvb{s}_{n4}", bufs=8)
                for i in range(CC6):
                    cc = (n4 + i) % CC6
                    lhsT = xT[cc][:, 128 * n4:128 * (n4 + 1)]
                    nc.tensor.matmul(pva[:], lhsT, vw[cc][:, 0:512],
                                     start=(i == 0), stop=(i == CC6 - 1))
                    nc.tensor.matmul(pvb[:], lhsT, vw[cc][:, 512:768],
                                     start=(i == 0), stop=(i == CC6 - 1))
                v3 = vsb[:].rearrange("p (h e) -> p h e", e=Dh + 1)
                cpy = nc.scalar.copy if s == 0 else nc.vector.tensor_copy
                cpy(v3[:, 0:8, 0:Dh], pva[:].rearrange("p (h e) -> p h e", e=Dh))
                cpy(v3[:, 8:12, 0:Dh], pvb[:].rearrange("p (h e) -> p h e", e=Dh))
                nc.vector.tensor_copy(v3[:, :, Dh:Dh + 1],
                                      onesf[:, 0:H].rearrange("p (h e) -> p h e", e=1))

            def emit_qk(s, jc):
                xT = xTs[s]
                qkt = get(qks[s], jc, lambda: sb.tile(
                    [128, N], F32R, tag="qk", name=f"qk_s{s}_{jc}", bufs=13))
                pqk = ps.tile([128, N], F32, tag="ps1", name=f"pqk{s}_{jc}", bufs=8)
                for i in range(CC6):
                    cc = (jc + i) % CC6
                    nc.tensor.matmul(pqk[:], qkw[cc][:, 128 * jc:128 * (jc + 1)], xT[cc][:],
                                     start=(i == 0), stop=(i == CC6 - 1))
                nc.vector.tensor_copy(qkt[:], pqk[:])

            def emit_head(s, h):
                qk, vsb = qks[s], vsbs[s]
                hb = 64 * (h % 2)
                qTh = qk[h // 2][hb:hb + 64, :]
                kTh = qk[CC6 + h // 2][hb:hb + 64, :]
                pts = []
                for mc in range(NC4):
                    pst = ps.tile([128, N], F32, tag="ps1", name=f"pst{s}_{h}_{mc}", bufs=8)
                    ptile = sb.tile([128, N], F32R, tag="pt", name=f"ptile{s}_{h}_{mc}", bufs=6)
                    if mc >= 2:
                        # mask added in-PSUM on DVE (PE/DVE load balance)
                        nc.tensor.matmul(pst[:], kTh[:, 128 * mc:128 * (mc + 1)], qTh,
                                         start=True, stop=True)
                        nc.vector.tensor_add(pst[:], pst[:], maskt[mc][:])
                    else:
                        # preload mask into PSUM (sets has_written), scores accumulate
                        nc.tensor.matmul(pst[:], ident[:], maskt[mc][:],
                                         start=True, stop=False, skip_group_check=True)
                        nc.tensor.matmul(pst[:], kTh[:, 128 * mc:128 * (mc + 1)], qTh,
                                         start=False, stop=True, skip_group_check=True)
                    nc.scalar.activation(ptile[:], pst[:],
                                         mybir.ActivationFunctionType.Exp)
                    pts.append(ptile)
                pot = ps.tile([Dh + 1, N], F32, tag="ps1", name=f"pot{s}_{h}", bufs=8)
                for mc in range(NC4):
                    nc.tensor.matmul(pot[:], vsb[mc][:, (Dh + 1) * h:(Dh + 1) * (h + 1)],
                                     pts[mc][:], start=(mc == 0), stop=(mc == NC4 - 1))
                recip = sb.tile([1, N], F32, tag="recip", name=f"recip{s}_{h}", bufs=3)
                nc.vector.reciprocal(recip[:], pot[Dh:Dh + 1, :])
                pbs = sb.tile([Dh, N], F32, tag="pbs", name=f"pbs{s}_{h}", bufs=3)
                nc.gpsimd.partition_broadcast(pbs[:], recip[:], channels=Dh)
                outT = get(outTs[s], h // 2, lambda: sb.tile(
                    [128, N], F32R, tag="outT", name=f"outT_s{s}_{h // 2}", bufs=10))
                with nc.allow_low_precision(reason="f32r outT"):
                    nc.vector.tensor_mul(outT[hb:hb + 64, :], pot[0:Dh, :], pbs[:])

            def emit_proj(s, n4):
                outT = outTs[s]
                if s == 1 and n4 == NC4 - 1:
                    # final unit: 3 narrow psum groups so the drain pipelines
                    osb = sb.tile([128, C], F32, tag="osb", name=f"osb{s}_{n4}", bufs=2)
                    for half in range(3):
                        c0 = 256 * half
                        pr = ps.tile([128, 256], F32, tag="ps1", name=f"pr{s}_{n4}_{half}", bufs=8)
                        for cc in range(CC6):
                            lhsT = outT[cc][:, 128 * n4:128 * (n4 + 1)]
                            nc.tensor.matmul(pr[:], lhsT, projw[cc][:, c0:c0 + 256],
                                             start=(cc == 0), stop=(cc == CC6 - 1))
                        eng = (nc.vector.tensor_copy, nc.scalar.copy)[half % 2]
                        eng(osb[:, c0:c0 + 256], pr[:])
                        deng = (nc.sync, nc.scalar)[half % 2]
                        deng.dma_start(y[s, 128 * n4:128 * (n4 + 1), c0:c0 + 256],
                                       osb[:, c0:c0 + 256])
                    return
                pra = ps.tile([128, 512], F32, tag="ps1", name=f"pra{s}_{n4}", bufs=8)
                prb = ps.tile([128, 256], F32, tag="ps1", name=f"prb{s}_{n4}", bufs=8)
                for cc in range(CC6):
                    lhsT = outT[cc][:, 128 * n4:128 * (n4 + 1)]
                    nc.tensor.matmul(pra[:], lhsT, projw[cc][:, 0:512],
                                     start=(cc == 0), stop=(cc == CC6 - 1))
                    nc.tensor.matmul(prb[:], lhsT, projw[cc][:, 512:768],
                                     start=(cc == 0), stop=(cc == CC6 - 1))
                osb = sb.tile([128, C], F32, tag="osb", name=f"osb{s}_{n4}", bufs=2)
                nc.vector.tensor_copy(osb[:, 0:512], pra[:])
                nc.sync.dma_start(y[s, 128 * n4:128 * (n4 + 1), 0:512], osb[:, 0:512])
                nc.scalar.copy(osb[:, 512:768], prb[:])
                nc.scalar.dma_start(y[s, 128 * n4:128 * (n4 + 1), 512:768], osb[:, 512:768])

            # ---- interleaved schedule ----
            for n4 in range(NC4):
                emit_transpose(0, n4)
            emit_weight_dmas()
            for n4 in range(NC4):
                emit_v(0, n4)
            for jc in range(2 * CC6):
                emit_qk(0, jc)
            # slice 0 attention interleaved with slice 1 early work
            e1 = [(emit_transpose, 1, n4) for n4 in range(NC4)] + \
                 [(emit_v, 1, n4) for n4 in range(NC4)] + \
                 [(emit_qk, 1, jc) for jc in range(2 * CC6)]
            k = 0
            for h in range(H):
                emit_head(0, h)
                if h == 3:
                    emit_projw_dmas()
                tgt = (len(e1) * (h + 1)) // H
                while k < tgt:
                    f, a, b = e1[k]; f(a, b); k += 1
            # slice 1 attention; slice 0 proj folded into the first heads
            p0 = [(emit_proj, 0, n4) for n4 in range(NC4)]
            k = 0
            for h in range(H):
                emit_head(1, h)
                if h < len(p0):
                    f, a, b = p0[k]; f(a, b); k += 1
            for n4 in range(NC4):
                emit_proj(1, n4)

    nc.finalize()
    return nc


def kernel(x, mask, qkv_w, q_bias, v_bias, proj_w, proj_b, _trace=False, _trace_kwargs=None):
    x, mask, qkv_w, proj_w = (np.asarray(a) for a in (x, mask, qkv_w, proj_w))
    q_bias, v_bias, proj_b = (np.asarray(a) for a in (q_bias, v_bias, proj_b))
    scale = Dh ** -0.5
    qkv_wT = np.ascontiguousarray(qkv_w.T).astype(np.float32)
    qkv_wT[:, :C] *= scale
    qkv_wTqk = np.ascontiguousarray(qkv_wT[:, :2 * C])
    qkv_wTv = np.ascontiguousarray(qkv_wT[:, 2 * C:])
    # biases folded in host-side only if nonzero (spec: all zeros). Assert to be safe.
    assert not np.any(q_bias) and not np.any(v_bias) and not np.any(proj_b), \
        "nonzero biases not supported by this kernel build"
    proj_wT = np.ascontiguousarray(proj_w.T).astype(np.float32)
    maskT = np.ascontiguousarray(mask.reshape(N, N).T).astype(np.float32)
    xf = np.ascontiguousarray(x.reshape(B * T, N, C)).astype(np.float32)

    if "nc" not in _cache:
        _cache["nc"] = build_nc()
    nc = _cache["nc"]

    in_maps = []
    for c in range(NCORES):
        in_maps.append({
            "xs": xf[SL * c:SL * (c + 1)],
            "qkv_wTqk": qkv_wTqk,
            "qkv_wTv": qkv_wTv,
            "proj_wT": proj_wT,
            "maskT": maskT,
        })
    res = run_bass_kernel_spmd(
        nc, in_maps, core_ids=list(range(NCORES)),
        trace=_trace, **(_trace_kwargs or {}),
    )
    out = np.concatenate([res.results[c]["y"] for c in range(NCORES)], axis=0)
    out = out.reshape(B, T, N, C)
    if _trace:
        return out, res
    return out


# revision 53
# speedup vs baseline: 1.0038x; 1.0038x over previous
"""Trainium2 Bass kernel for nn_Attention (B=2,T=8,N=512,C=768,H=12).

Strategy: data-parallel over the 16 (b,t) slices -> 2 slices per core, 8 cores.
All weight/mask transposes are done on host. On-chip per slice:
  xT = transpose(x)                      (PE transposes, 128x128 tiles)
  qkvT[q,k] = W_qk @ xT                  ([d, n] layout; scale folded into Wq on host)
  v = xT.T @ W_v                         ([token, d] layout)
  ST[m,n] = kT.T @ qT ; P = exp(ST + maskT)   (no max-subtraction: logits bounded)
  OT'[0:64] = v_h.T @ P (PV), OT'[64] = row-sums l (ones column in v tiles)
  outT[c,n] = OT' * broadcast(1/l)       (ones-matmul broadcast of recip row)
  y = outT.T @ proj_wT                   ([n, C] layout, DMA out)
All matmuls run in float32r (full PE rate at N>=256).
"""
import sys

sys.path.insert(0, "/opt/trn_rl_repo")

import numpy as np
import concourse.bacc as bacc
import concourse.mybir as mybir
import concourse.tile as tile
from concourse.bass_utils import run_bass_kernel_spmd
from concourse.masks import make_identity

B, T, N, C = 2, 8, 512, 768
H = 12
Dh = C // H            # 64
SL = 2                 # slices per core
NCORES = 8
NC4 = N // 128         # 4 n-chunks
CC6 = C // 128         # 6 c-chunks
F32 = mybir.dt.float32
F32R = mybir.dt.float32r

_cache = {}


def build_nc():
    nc = bacc.Bacc()
    xs = nc.dram_tensor("xs", [SL, N, C], F32R, kind="ExternalInput")
    qkv_wTqk = nc.dram_tensor("qkv_wTqk", [C, 2 * C], F32R, kind="ExternalInput")
    qkv_wTv = nc.dram_tensor("qkv_wTv", [C, C], F32R, kind="ExternalInput")
    proj_wT = nc.dram_tensor("proj_wT", [C, C], F32R, kind="ExternalInput")
    maskT = nc.dram_tensor("maskT", [N, N], F32R, kind="ExternalInput")
    y = nc.dram_tensor("y", [SL, N, C], F32, kind="ExternalOutput")

    with tile.TileContext(nc) as tc:
        with (
            tc.tile_pool(name="wpool", bufs=1) as wpool,
            tc.tile_pool(name="sb", bufs=1) as sb,
            tc.tile_pool(name="ps", bufs=1, space="PSUM") as ps,
        ):
            # ---- persistent weights ----
            qkw = [wpool.tile([128, 2 * C], F32R, tag=f"qkw{cc}", name=f"qkw{cc}") for cc in range(CC6)]
            vw = [wpool.tile([128, C], F32R, tag=f"vw{cc}", name=f"vw{cc}") for cc in range(CC6)]
            projw = [wpool.tile([128, C], F32R, tag=f"projw{cc}", name=f"projw{cc}") for cc in range(CC6)]
            maskt = [wpool.tile([128, N], F32R, tag=f"maskt{mc}", name=f"maskt{mc}") for mc in range(NC4)]
            def emit_weight_dmas():
                for cc in range(CC6):
                    eng = nc.gpsimd if cc % 2 == 0 else nc.scalar
                    eng.dma_start(vw[cc][:], qkv_wTv[128 * cc:128 * (cc + 1), :])
                for cc in range(CC6):
                    eng = (nc.gpsimd, nc.scalar)[cc % 2]
                    eng.dma_start(qkw[cc][:], qkv_wTqk[128 * cc:128 * (cc + 1), :])
                for mc in range(NC4):
                    nc.sync.dma_start(maskt[mc][:], maskT[128 * mc:128 * (mc + 1), :])

            def emit_projw_dmas():
                for cc in range(CC6):
                    nc.gpsimd.dma_start(projw[cc][:], proj_wT[128 * cc:128 * (cc + 1), :])
            identf = wpool.tile([128, 128], F32, tag="identf")
            make_identity(nc, identf[:])
            ident = wpool.tile([128, 128], F32R, tag="ident")
            nc.vector.tensor_copy(ident[:], identf[:])
            onesf = wpool.tile([128, Dh], F32, tag="onesf")
            nc.gpsimd.memset(onesf[:], 1.0)

            xTs = [[None] * CC6 for _ in range(SL)]
            vsbs = [[None] * NC4 for _ in range(SL)]
            qks = [[None] * (2 * CC6) for _ in range(SL)]
            outTs = [[None] * CC6 for _ in range(SL)]

            def get(lst, i, mk):
                if lst[i] is None:
                    lst[i] = mk()
                return lst[i]

            def emit_transpose(s, n4):
                # one contiguous block DMA, then transpose 6 column chunks
                xblk = sb.tile([128, C], F32R, tag="xin", name=f"xblk{s}_{n4}", bufs=3)
                if s == 0 and n4 == 0:
                    # split the very first block so transposes start earlier
                    nc.sync.dma_start(xblk[:, 0:384], xs[s, 0:128, 0:384])
                    nc.sync.dma_start(xblk[:, 384:C], xs[s, 0:128, 384:C])
                else:
                    nc.sync.dma_start(xblk[:], xs[s, 128 * n4:128 * (n4 + 1), :])
                for cc in range(CC6):
                    xT = get(xTs[s], cc, lambda cc=cc: sb.tile(
                        [128, N], F32R, tag="xT", name=f"xT_s{s}_{cc}", bufs=8))
                    pt = ps.tile([128, 128], F32R, tag="ps1", name=f"pt{s}_{n4}_{cc}", bufs=8)
                    nc.tensor.transpose(pt[:], xblk[:, 128 * cc:128 * (cc + 1)], ident[:])
                    nc.vector.tensor_copy(xT[:, 128 * n4:128 * (n4 + 1)], pt[:])

            def emit_v(s, n4):
                xT = xTs[s]
                vsb = get(vsbs[s], n4, lambda: sb.tile(
                    [128, H * (Dh + 1)], F32R, tag="vsb", name=f"vsb_s{s}_{n4}", bufs=8))
                pva = ps.tile([128, 512], F32, tag="ps1", name=f"pva{s}_{n4}", bufs=8)
                pvb = ps.tile([128, 256], F32, tag="ps1", name=f"pvb{s}_{n4}", bufs=8)
                for i in range(CC6):
                    cc = (n4 + i) % CC6
                    lhsT = xT[cc][:, 128 * n4:128 * (n4 + 1)]
                    nc.tensor.matmul(pva[:], lhsT, vw[cc][:, 0:512],
                                     start=(i == 0), stop=(i == CC6 - 1))
                    nc.tensor.matmul(pvb[:], lhsT, vw[cc][:, 512:768],
                                     start=(i == 0), stop=(i == CC6 - 1))
                v3 = vsb[:].rearrange("p (h e) -> p h e", e=Dh + 1)
                cpy = nc.scalar.copy if s == 0 else nc.vector.tensor_copy
                cpy(v3[:, 0:8, 0:Dh], pva[:].rearrange("p (h e) -> p h e", e=Dh))
                cpy(v3[:, 8:12, 0:Dh], pvb[:].rearrange("p (h e) -> p h e", e=Dh))
                nc.vector.tensor_copy(v3[:, :, Dh:Dh + 1],
                                      onesf[:, 0:H].rearrange("p (h e) -> p h e", e=1))

            def emit_qk(s, jc):
                xT = xTs[s]
                qkt = get(qks[s], jc, lambda: sb.tile(
                    [128, N], F32R, tag="qk", name=f"qk_s{s}_{jc}", bufs=13))
                pqk = ps.tile([128, N], F32, tag="ps1", name=f"pqk{s}_{jc}", bufs=8)
                for i in range(CC6):
                    cc = (jc + i) % CC6
                    nc.tensor.matmul(pqk[:], qkw[cc][:, 128 * jc:128 * (jc + 1)], xT[cc][:],
                                     start=(i == 0), stop=(i == CC6 - 1))
                nc.vector.tensor_copy(qkt[:], pqk[:])

            def emit_head(s, h):
                qk, vsb = qks[s], vsbs[s]
                hb = 64 * (h % 2)
                qTh = qk[h // 2][hb:hb + 64, :]
                kTh = qk[CC6 + h // 2][hb:hb + 64, :]
                pts = []
                for mc in range(NC4):
                    pst = ps.tile([128, N], F32, tag="ps1", name=f"pst{s}_{h}_{mc}", bufs=8)
                    ptile = sb.tile([128, N], F32R, tag="pt", name=f"ptile{s}_{h}_{mc}", bufs=6)
                    if mc >= 2:
                        # mask added in-PSUM on DVE (PE/DVE load balance)
                        nc.tensor.matmul(pst[:], kTh[:, 128 * mc:128 * (mc + 1)], qTh,
                                         start=True, stop=True)
                        nc.vector.tensor_add(pst[:], pst[:], maskt[mc][:])
                    else:
                        # preload mask into PSUM (sets has_written), scores accumulate
                        nc.tensor.matmul(pst[:], ident[:], maskt[mc][:],
                                         start=True, stop=False, skip_group_check=True)
                        nc.tensor.matmul(pst[:], kTh[:, 128 * mc:128 * (mc + 1)], qTh,
                                         start=False, stop=True, skip_group_check=True)
                    nc.scalar.activation(ptile[:], pst[:],
                                         mybir.ActivationFunctionType.Exp)
                    pts.append(ptile)
                pot = ps.tile([Dh + 1, N], F32, tag="ps1", name=f"pot{s}_{h}", bufs=8)
                for mc in range(NC4):
                    nc.tensor.matmul(pot[:], vsb[mc][:, (Dh + 1) * h:(Dh + 1) * (h + 1)],
                                     pts[mc][:], start=(mc == 0), stop=(mc == NC4 - 1))
                recip = sb.tile([1, N], F32, tag="recip", name=f"recip{s}_{h}", bufs=3)
                nc.vector.reciprocal(recip[:], pot[Dh:Dh + 1, :])
                pbs = sb.tile([Dh, N], F32, tag="pbs", name=f"pbs{s}_{h}", bufs=3)
                nc.gpsimd.partition_broadcast(pbs[:], recip[:], channels=Dh)
                outT = get(outTs[s], h // 2, lambda: sb.tile(
                    [128, N], F32R, tag="outT", name=f"outT_s{s}_{h // 2}", bufs=10))
                with nc.allow_low_precision(reason="f32r outT"):
                    nc.vector.tensor_mul(outT[hb:hb + 64, :], pot[0:Dh, :], pbs[:])

            def emit_proj(s, n4):
                outT = outTs[s]
                if s == 1 and n4 == NC4 - 1:
                    # final unit: 3 narrow psum groups so the drain pipelines
                    osb = sb.tile([128, C], F32, tag="osb", name=f"osb{s}_{n4}", bufs=2)
                    for half in range(3):
                        c0 = 256 * half
                        pr = ps.tile([128, 256], F32, tag="ps1", name=f"pr{s}_{n4}_{half}", bufs=8)
                        for cc in range(CC6):
                            lhsT = outT[cc][:, 128 * n4:128 * (n4 + 1)]
                            nc.tensor.matmul(pr[:], lhsT, projw[cc][:, c0:c0 + 256],
                                             start=(cc == 0), stop=(cc == CC6 - 1))
                        eng = (nc.vector.tensor_copy, nc.scalar.copy)[half % 2]
                        eng(osb[:, c0:c0 + 256], pr[:])
                        deng = (nc.sync, nc.scalar)[half % 2]
                        deng.dma_start(y[s, 128 * n4:128 * (n4 + 1), c0:c0 + 256],
                                       osb[:, c0:c0 + 256])
                    return
                pra = ps.tile([128, 512], F32, tag="ps1", name=f"pra{s}_{n4}", bufs=8)
                prb = ps.tile([128, 256], F32, tag="ps1", name=f"prb{s}_{n4}", bufs=8)
                for cc in range(CC6):
                    lhsT = outT[cc][:, 128 * n4:128 * (n4 + 1)]
                    nc.tensor.matmul(pra[:], lhsT, projw[cc][:, 0:512],
                                     start=(cc == 0), stop=(cc == CC6 - 1))
                    nc.tensor.matmul(prb[:], lhsT, projw[cc][:, 512:768],
                                     start=(cc == 0), stop=(cc == CC6 - 1))
                osb = sb.tile([128, C], F32, tag="osb", name=f"osb{s}_{n4}", bufs=2)
                nc.vector.tensor_copy(osb[:, 0:512], pra[:])
                nc.sync.dma_start(y[s, 128 * n4:128 * (n4 + 1), 0:512], osb[:, 0:512])
                nc.scalar.copy(osb[:, 512:768], prb[:])
                nc.scalar.dma_start(y[s, 128 * n4:128 * (n4 + 1), 512:768], osb[:, 512:768])

            # ---- interleaved schedule ----
            for n4 in range(NC4):
                emit_transpose(0, n4)
            emit_weight_dmas()
            for n4 in range(NC4):
                emit_v(0, n4)
            for jc in range(2 * CC6):
                emit_qk(0, jc)
            # slice 0 attention interleaved with slice 1 early work
            e1 = [(emit_transpose, 1, n4) for n4 in range(NC4)] + \
                 [(emit_v, 1, n4) for n4 in range(NC4)] + \
                 [(emit_qk, 1, jc) for jc in range(2 * CC6)]
            k = 0
            for h in range(H):
                emit_head(0, h)
                if h == 3:
                    emit_projw_dmas()
                tgt = (len(e1) * (h + 1)) // H
                while k < tgt:
                    f, a, b = e1[k]; f(a, b); k += 1
            # slice 1 attention; slice 0 proj folded into the first heads
            p0 = [(emit_proj, 0, n4) for n4 in range(NC4)]
            k = 0
            for h in range(H):
                emit_head(1, h)
                if h < len(p0):
                    f, a, b = p0[k]; f(a, b); k += 1
            for n4 in range(NC4):
                emit_proj(1, n4)

    nc.finalize()
    return nc


def kernel(x, mask, qkv_w, q_bias, v_bias, proj_w, proj_b, _trace=False, _trace_kwargs=None):
    x, mask, qkv_w, proj_w = (np.asarray(a) for a in (x, mask, qkv_w, proj_w))
    q_bias, v_bias, proj_b = (np.asarray(a) for a in (q_bias, v_bias, proj_b))
    scale = Dh ** -0.5
    qkv_wT = np.ascontiguousarray(qkv_w.T).astype(np.float32)
    qkv_wT[:, :C] *= scale
    qkv_wTqk = np.ascontiguousarray(qkv_wT[:, :2 * C])
    qkv_wTv = np.ascontiguousarray(qkv_wT[:, 2 * C:])
    # biases folded in host-side only if nonzero (spec: all zeros). Assert to be safe.
    assert not np.any(q_bias) and not np.any(v_bias) and not np.any(proj_b), \
        "nonzero biases not supported by this kernel build"
    proj_wT = np.ascontiguousarray(proj_w.T).astype(np.float32)
    maskT = np.ascontiguousarray(mask.reshape(N, N).T).astype(np.float32)
    xf = np.ascontiguousarray(x.reshape(B * T, N, C)).astype(np.float32)

    if "nc" not in _cache:
        _cache["nc"] = build_nc()
    nc = _cache["nc"]

    in_maps = []
    for c in range(NCORES):
        in_maps.append({
            "xs": xf[SL * c:SL * (c + 1)],
            "qkv_wTqk": qkv_wTqk,
            "qkv_wTv": qkv_wTv,
            "proj_wT": proj_wT,
            "maskT": maskT,
        })
    res = run_bass_kernel_spmd(
        nc, in_maps, core_ids=list(range(NCORES)),
        trace=_trace, **(_trace_kwargs or {}),
    )
    out = np.concatenate([res.results[c]["y"] for c in range(NCORES)], axis=0)
    out = out.reshape(B, T, N, C)
    if _trace:
        return out, res
    return out
